# revision 52
# baseline (speedup 1.0000x reference)
"""Trainium2 Bass kernel for nn_KermtDualCausalCv4 (dual-trunk dense MLP).

Strategy:
  * Layer-0 algebraic compression: the [T, 12325] input features are
    structured (broadcast mol_vec, rank-1 solvent blocks s_j[t]*v_j,
    desc_seq repeated 6x, prev/next rf scalar). Folding the structure into
    W0 on the host turns the [H, 12325] first layer into an effective
    [H, 12] matmul plus a per-hidden-unit constant (absorbed into the bias):
        h0[t] = W0eff @ f[t] + c,   f[t] = [solvent_seq[t], desc_seq[t], rf_nb[t]]
    This removes ~413 of the 551 GFLOP the reference performs.
  * 8-way data parallelism over the 4096 tokens (512 tokens per core);
    every core runs both trunks end-to-end. No collectives.
  * All matmuls in f32r (full fp32 data, TensorE 1 cycle/row at N=512,
    ~1.5e-4 relative error per matmul measured on HW).
  * Weights pre-tiled on the host into [mo][ki][ko*mi] slabs so each
    weight DMA is 128 partitions x 8KB contiguous (1 MiB per transfer).
  * Exact (erf-based) GELU + per-partition bias fused into the PSUM->SBUF
    eviction on the scalar (ACT) engine.
"""

import sys
import numpy as np

for _p in ("/opt/trn_rl_repo",):
    if _p not in sys.path:
        sys.path.append(_p)

import concourse.bass as bass  # noqa: E402
import concourse.mybir as mybir  # noqa: E402
import concourse.tile as tile  # noqa: E402
from concourse import bacc  # noqa: E402
from concourse.bass_utils import run_bass_kernel_spmd  # noqa: E402

T = 4096
MOL = 2048
DESC = 6
NSOL = 5
H = 2048
D_BLOCK = MOL + DESC  # 2054
D_IN = D_BLOCK * (1 + NSOL) + 1  # 12325
BOUNDARY_RF = 0.5
CLAMP_LO, CLAMP_HI = 1e-4, 1.0 - 1e-4

N_CORES = 8
TT = T // N_CORES  # 512 tokens per core
KT = H // 128  # 16 k/m tiles per hidden dim
F_DIM = 16  # padded effective feature count (12 used)

F32 = mybir.dt.float32
F32R = mybir.dt.float32r
GELU = mybir.ActivationFunctionType.Gelu
COPY = mybir.ActivationFunctionType.Identity

_CACHE: dict = {}


def _build_nc():
    """Emit the Bass/Tile kernel (identical program for all 8 cores)."""
    nc = bacc.Bacc(None, target_bir_lowering=False)

    dram = {}
    for tr in ("f", "b"):
        # feat/w0 are zero-padded to K=128 so layer-0 matmuls are standard
        # full-contraction matmuls (PE cost is N cycles regardless of K, and
        # LDWEIGHTS pipelines like the main layers).
        dram[f"feat_{tr}"] = nc.dram_tensor(f"feat_{tr}", [128, TT], F32R,
                                            kind="ExternalInput")
        dram[f"w0_{tr}"] = nc.dram_tensor(f"w0_{tr}", [128, H], F32R,
                                          kind="ExternalInput")
        for ly in (1, 2):
            dram[f"w{ly}_{tr}"] = nc.dram_tensor(f"w{ly}_{tr}", [KT, 128, H],
                                                 F32R, kind="ExternalInput")
        dram[f"wh_{tr}"] = nc.dram_tensor(f"wh_{tr}", [128, KT], F32R,
                                          kind="ExternalInput")
        for ly in (1, 2):
            dram[f"b{ly}_{tr}"] = nc.dram_tensor(f"b{ly}_{tr}", [128, KT], F32,
                                                 kind="ExternalInput")
        dram[f"bh_{tr}"] = nc.dram_tensor(f"bh_{tr}", [1, 1], F32,
                                          kind="ExternalInput")
        dram[f"rf_{tr}"] = nc.dram_tensor(f"rf_{tr}", [1, TT], F32,
                                          kind="ExternalOutput")

    with tile.TileContext(nc) as tc:
        with (
            tc.tile_pool(name="const", bufs=1) as const,
            tc.tile_pool(name="acts", bufs=3) as acts,
            tc.tile_pool(name="wpool", bufs=5) as wpool,
            tc.tile_pool(name="psum", bufs=5, space="PSUM") as psum,
            tc.tile_pool(name="psum_h", bufs=2, space="PSUM") as psum_h,
            tc.tile_pool(name="outp", bufs=2) as outp,
        ):
            cst = {}

            def load_const(nm, shp, dt):
                t = const.tile(shp, dt, tag=nm)
                nc.sync.dma_start(t[:], dram[nm][:])
                cst[nm] = t

            def load_trunk_consts(tr, critical=True):
                if critical:
                    # feat/w0 gate the first layer-0 matmul: load them first.
                    # (layer-0 bias rides in the matmul via the ones-feature.)
                    if tr == "f":
                        # GPSIMD SWDGE descriptor-gen starts earlier than the
                        # sync queue's; split w0 so the first matmuls only
                        # wait for their slice.
                        tf = const.tile([128, TT], F32R, tag=f"feat_{tr}",
                                        name=f"feat_{tr}")
                        nc.gpsimd.dma_start(tf[:], dram[f"feat_{tr}"][:])
                        cst[f"feat_{tr}"] = tf
                        t = const.tile([128, H], F32R, tag=f"w0_{tr}",
                                       name=f"w0_{tr}")
                        nc.gpsimd.dma_start(t[:, 0:256], dram[f"w0_{tr}"][:, 0:256])
                        nc.gpsimd.dma_start(t[:, 256:H], dram[f"w0_{tr}"][:, 256:H])
                        cst[f"w0_{tr}"] = t
                    else:
                        load_const(f"feat_{tr}", [128, TT], F32R)
                        load_const(f"w0_{tr}", [128, H], F32R)
                else:
                    for ly in (1, 2):
                        load_const(f"b{ly}_{tr}", [128, KT], F32)
                    load_const(f"wh_{tr}", [128, KT], F32R)
                    load_const(f"bh_{tr}", [1, 1], F32)

            def layer0_mm(tr, g0, mo):
                ps = psum.tile([128, TT], F32, tag="ps")
                nc.tensor.matmul(ps[:], cst[f"w0_{tr}"][:, mo * 128:(mo + 1) * 128],
                                 cst[f"feat_{tr}"][:],
                                 start=True, stop=True)
                nc.scalar.activation(g0[:, mo, :], ps[:], GELU)

            def layer0(tr, tag="acts", bufs=None):
                kw = {} if bufs is None else {"bufs": bufs}
                g0 = acts.tile([128, KT, TT], F32R, tag=tag, **kw)
                for mo in range(KT):
                    layer0_mm(tr, g0, mo)
                return g0

            def head_mm(tr, psh, ko):
                nc.tensor.matmul(psh[:], cst[f"wh_{tr}"][:, ko:ko + 1],
                                 g2s[tr][:, ko, :],
                                 start=(ko == 0), stop=(ko == KT - 1))

            def chain(ps, wslab, g_in, ko_rng, start, stop):
                for ko in ko_rng:
                    nc.tensor.matmul(ps[:], wslab[:, ko * 128:(ko + 1) * 128],
                                     g_in[:, ko, :],
                                     start=(start and ko == ko_rng[0]),
                                     stop=(stop and ko == ko_rng[-1]))

            def load_slab(ly, tr, mo):
                wslab = wpool.tile([128, H], F32R, tag="wslab")
                nc.sync.dma_start(wslab[:], dram[f"w{ly}_{tr}"][mo, :, :])
                return wslab

            def evict(g_out, tr, ly, mo, ps):
                nc.scalar.activation(g_out[:, mo, :], ps[:], GELU,
                                     bias=cst[f"b{ly}_{tr}"][:, mo:mo + 1])

            def layer(tr, ly, g_in, head=False, il_l0=None, warm=0,
                      after_warm=None):
                # head: interleave this trunk's head matmuls (lag 2).
                # il_l0: (trunk, g0) whose layer-0 work rides along this layer.
                # warm: start this many half-chains before g_in fully evicted.
                g_out = acts.tile([128, KT, TT], F32R, tag="acts")
                if head:
                    g2s[tr] = g_out
                    psh = psum_h.tile([1, TT], F32, tag="psh")
                KH = KT // 2
                pend = []
                for mo in range(warm):
                    wslab = load_slab(ly, tr, mo)
                    ps = psum.tile([128, TT], F32, tag="ps")
                    chain(ps, wslab, g_in, range(KH), start=True, stop=False)
                    pend.append((mo, wslab, ps))
                if after_warm is not None:
                    after_warm()
                for mo, wslab, ps in pend:
                    chain(ps, wslab, g_in, range(KH, KT), start=False, stop=True)
                    evict(g_out, tr, ly, mo, ps)
                il_pend = list(range(KT)) if il_l0 is not None else []
                for mo in range(warm, KT):
                    wslab = load_slab(ly, tr, mo)
                    if head and mo >= 2:
                        head_mm(tr, psh, mo - 2)
                    n_il = -(-len(il_pend) // max(1, KT - mo))  # spread evenly
                    for _ in range(n_il):
                        layer0_mm(il_l0[0], il_l0[1], il_pend.pop(0))
                    ps = psum.tile([128, TT], F32, tag="ps")
                    chain(ps, wslab, g_in, range(KT), start=True, stop=True)
                    evict(g_out, tr, ly, mo, ps)
                if head:
                    head_mm(tr, psh, KT - 2)
                    head_mm(tr, psh, KT - 1)
                    rf_sb = outp.tile([1, TT], F32, tag="rf")
                    nc.scalar.activation(rf_sb[:], psh[:], COPY,
                                         bias=cst[f"bh_{tr}"][:1, :1])
                    nc.vector.tensor_scalar(rf_sb[:], rf_sb[:], CLAMP_LO, CLAMP_HI,
                                            op0=mybir.AluOpType.max,
                                            op1=mybir.AluOpType.min)
                    nc.sync.dma_start(dram[f"rf_{tr}"][:], rf_sb[:])
                return g_out

            g2s = {}
            # Pre-warm the PE clock: dependency-free dummy matmuls sized to
            # finish right as layer 0's operands land from HBM.
            warmup = const.tile([128, 64], F32, tag="warmup")
            nc.vector.memset(warmup[:], 0.0)
            ps_w = psum_h.tile([1, TT], F32, tag="psh")
            for _ in range(10):
                nc.tensor.matmul(ps_w[:, 0:64], warmup[:, 0:1], warmup[:],
                                 start=True, stop=True)
            load_trunk_consts("f")
            g0f = layer0("f")
            # g0b has its own slot: it stays live across the whole fwd trunk
            g0b = acts.tile([128, KT, TT], F32R, tag="acts_b0", bufs=1)

            def _deferred_consts():
                load_trunk_consts("f", critical=False)
                load_trunk_consts("b")
                load_trunk_consts("b", critical=False)

            g1f = layer("f", 1, g0f, il_l0=("b", g0b), warm=4,
                        after_warm=_deferred_consts)
            layer("f", 2, g1f, head=True)
            g1b = layer("b", 1, g0b)
            layer("b", 2, g1b, head=True)

    nc.compile()
    return nc


def _compress_w0(W0, b0, mol_vec, solvent_vecs):
    """Fold the feature structure into W0: return (W0effT [F_DIM, H], c [H])."""
    W0 = W0.astype(np.float64)
    cols = []
    for j in range(NSOL):
        off = D_BLOCK * (1 + j)
        cols.append(W0[:, off:off + MOL] @ solvent_vecs[j].astype(np.float64))
    A = W0[:, MOL:MOL + DESC].copy()
    for j in range(NSOL):
        off = D_BLOCK * (1 + j) + MOL
        A += W0[:, off:off + DESC]
    for d in range(DESC):
        cols.append(A[:, d])
    cols.append(W0[:, D_IN - 1])  # prev/next rf column
    W0eff = np.stack(cols, axis=1)  # [H, 12]
    c = W0[:, :MOL] @ mol_vec.astype(np.float64) + b0.astype(np.float64)
    W0effT = np.zeros((F_DIM, H), np.float32)
    W0effT[:W0eff.shape[1], :] = W0eff.T.astype(np.float32)
    return W0effT, c.astype(np.float32)


def _tile_w(W):
    """[H_out, H_in] -> [mo, ki, ko*mi] slabs, lhsT[k, m] = W[m, k]."""
    a = W.reshape(KT, 128, KT, 128)  # [mo, mi, ko, ki]
    return np.ascontiguousarray(a.transpose(0, 3, 2, 1)).reshape(KT, 128, H)


def _part_major(v):
    """[H] -> [128, KT] with v[mo*128+p] at [p, mo]."""
    return np.ascontiguousarray(v.reshape(KT, 128).T)


def kernel(mol_vec, solvent_seq, desc_seq, rf_true, solvent_vecs,
           Wf0, bf0, Wf1, bf1, Wf2, bf2, Wof, bof,
           Wb0, bb0, Wb1, bb1, Wb2, bb2, Wob, bob):
    args = {k: np.asarray(v, np.float32) for k, v in locals().items()}
    mol_vec = args["mol_vec"]; solvent_seq = args["solvent_seq"]
    desc_seq = args["desc_seq"]; rf_true = args["rf_true"]
    solvent_vecs = args["solvent_vecs"]

    prev = np.concatenate([[np.float32(BOUNDARY_RF)], rf_true[:-1]])
    nxt = np.concatenate([rf_true[1:], [np.float32(BOUNDARY_RF)]])

    shared = {}
    for tr, W0, b0, W1, b1, W2, b2, Wh, bh, rf_nb in (
        ("f", args["Wf0"], args["bf0"], args["Wf1"], args["bf1"],
         args["Wf2"], args["bf2"], args["Wof"], args["bof"], prev),
        ("b", args["Wb0"], args["bb0"], args["Wb1"], args["bb1"],
         args["Wb2"], args["bb2"], args["Wob"], args["bob"], nxt),
    ):
        W0effT, c = _compress_w0(W0, b0, mol_vec, solvent_vecs)
        W0effT[NSOL + DESC + 1, :] = c  # bias via the ones-feature row
        w0p = np.zeros((128, H), np.float32)
        w0p[:F_DIM] = W0effT
        shared[f"w0_{tr}"] = w0p
        shared[f"w1_{tr}"] = _tile_w(W1)
        shared[f"b1_{tr}"] = _part_major(b1)
        shared[f"w2_{tr}"] = _tile_w(W2)
        shared[f"b2_{tr}"] = _part_major(b2)
        shared[f"wh_{tr}"] = np.ascontiguousarray(Wh.reshape(KT, 128).T)
        shared[f"bh_{tr}"] = bh.reshape(1, 1)
        feat = np.zeros((128, T), np.float32)
        feat[0:NSOL, :] = solvent_seq.T
        feat[NSOL:NSOL + DESC, :] = desc_seq.T
        feat[NSOL + DESC, :] = rf_nb
        feat[NSOL + DESC + 1, :] = 1.0  # ones row carries the layer-0 bias
        shared[f"feat_{tr}"] = feat

    if "nc" not in _CACHE:
        _CACHE["nc"] = _build_nc()
    nc = _CACHE["nc"]

    in_maps = []
    for core in range(N_CORES):
        m = {k: v for k, v in shared.items() if not k.startswith("feat_")}
        sl = slice(core * TT, (core + 1) * TT)
        m["feat_f"] = np.ascontiguousarray(shared["feat_f"][:, sl])
        m["feat_b"] = np.ascontiguousarray(shared["feat_b"][:, sl])
        in_maps.append(m)

    res = run_bass_kernel_spmd(nc, in_maps, core_ids=list(range(N_CORES)))
    rf_fwd = np.concatenate([res.results[c]["rf_f"][0] for c in range(N_CORES)])
    rf_bwd = np.concatenate([res.results[c]["rf_b"][0] for c in range(N_CORES)])
    return rf_fwd.astype(np.float32), rf_bwd.astype(np.float32)


# revision 53
# speedup vs baseline: 1.0537x; 1.0537x over previous
"""Trainium2 Bass kernel for nn_KermtDualCausalCv4 (dual-trunk dense MLP).

Strategy:
  * Layer-0 algebraic compression: the [T, 12325] input features are
    structured (broadcast mol_vec, rank-1 solvent blocks s_j[t]*v_j,
    desc_seq repeated 6x, prev/next rf scalar). Folding the structure into
    W0 on the host turns the [H, 12325] first layer into an effective
    [H, 12] matmul plus a per-hidden-unit constant (absorbed into the bias):
        h0[t] = W0eff @ f[t] + c,   f[t] = [solvent_seq[t], desc_seq[t], rf_nb[t]]
    This removes ~413 of the 551 GFLOP the reference performs.
  * 8-way data parallelism over the 4096 tokens (512 tokens per core);
    every core runs both trunks end-to-end. No collectives.
  * All matmuls in f32r (full fp32 data, TensorE 1 cycle/row at N=512,
    ~1.5e-4 relative error per matmul measured on HW).
  * Weights pre-tiled on the host into [mo][ki][ko*mi] slabs so each
    weight DMA is 128 partitions x 8KB contiguous (1 MiB per transfer).
  * Exact (erf-based) GELU + per-partition bias fused into the PSUM->SBUF
    eviction on the scalar (ACT) engine.
"""

import sys
import numpy as np

for _p in ("/opt/trn_rl_repo",):
    if _p not in sys.path:
        sys.path.append(_p)

import concourse.bass as bass  # noqa: E402
import concourse.mybir as mybir  # noqa: E402
import concourse.tile as tile  # noqa: E402
from concourse import bacc  # noqa: E402
from concourse.bass_utils import run_bass_kernel_spmd  # noqa: E402

T = 4096
MOL = 2048
DESC = 6
NSOL = 5
H = 2048
D_BLOCK = MOL + DESC  # 2054
D_IN = D_BLOCK * (1 + NSOL) + 1  # 12325
BOUNDARY_RF = 0.5
CLAMP_LO, CLAMP_HI = 1e-4, 1.0 - 1e-4

N_CORES = 8
TT = T // N_CORES  # 512 tokens per core
KT = H // 128  # 16 k/m tiles per hidden dim
F_DIM = 16  # padded effective feature count (12 used)

F32 = mybir.dt.float32
F32R = mybir.dt.float32r
GELU = mybir.ActivationFunctionType.Gelu
COPY = mybir.ActivationFunctionType.Identity

_CACHE: dict = {}


def _build_nc():
    """Emit the Bass/Tile kernel (identical program for all 8 cores)."""
    nc = bacc.Bacc(None, target_bir_lowering=False)

    dram = {}
    for tr in ("f", "b"):
        # feat/w0 are zero-padded to K=128 so layer-0 matmuls are standard
        # full-contraction matmuls (PE cost is N cycles regardless of K, and
        # LDWEIGHTS pipelines like the main layers).
        dram[f"feat_{tr}"] = nc.dram_tensor(f"feat_{tr}", [128, TT], F32R,
                                            kind="ExternalInput")
        dram[f"w0_{tr}"] = nc.dram_tensor(f"w0_{tr}", [128, H], F32R,
                                          kind="ExternalInput")
        for ly in (1, 2):
            dram[f"w{ly}_{tr}"] = nc.dram_tensor(f"w{ly}_{tr}", [KT, 128, H],
                                                 F32R, kind="ExternalInput")
        dram[f"wh_{tr}"] = nc.dram_tensor(f"wh_{tr}", [128, KT], F32R,
                                          kind="ExternalInput")
        for ly in (1, 2):
            dram[f"b{ly}_{tr}"] = nc.dram_tensor(f"b{ly}_{tr}", [128, KT], F32,
                                                 kind="ExternalInput")
        dram[f"bh_{tr}"] = nc.dram_tensor(f"bh_{tr}", [1, 1], F32,
                                          kind="ExternalInput")
        dram[f"rf_{tr}"] = nc.dram_tensor(f"rf_{tr}", [1, TT], F32,
                                          kind="ExternalOutput")

    with tile.TileContext(nc) as tc:
        with (
            tc.tile_pool(name="const", bufs=1) as const,
            tc.tile_pool(name="acts", bufs=3) as acts,
            tc.tile_pool(name="wpool", bufs=5) as wpool,
            tc.tile_pool(name="psum", bufs=5, space="PSUM") as psum,
            tc.tile_pool(name="psum_h", bufs=2, space="PSUM") as psum_h,
            tc.tile_pool(name="outp", bufs=2) as outp,
        ):
            cst = {}

            def load_const(nm, shp, dt):
                t = const.tile(shp, dt, tag=nm)
                nc.sync.dma_start(t[:], dram[nm][:])
                cst[nm] = t

            def load_trunk_consts(tr, critical=True):
                if critical:
                    # feat/w0 gate the first layer-0 matmul: load them first.
                    # (layer-0 bias rides in the matmul via the ones-feature.)
                    load_const(f"feat_{tr}", [128, TT], F32R)
                    if tr == "f":
                        # split so the first matmuls only wait for their slice
                        t = const.tile([128, H], F32R, tag=f"w0_{tr}",
                                       name=f"w0_{tr}")
                        nc.sync.dma_start(t[:, 0:256], dram[f"w0_{tr}"][:, 0:256])
                        nc.sync.dma_start(t[:, 256:H], dram[f"w0_{tr}"][:, 256:H])
                        cst[f"w0_{tr}"] = t
                    else:
                        load_const(f"w0_{tr}", [128, H], F32R)
                else:
                    for ly in (1, 2):
                        load_const(f"b{ly}_{tr}", [128, KT], F32)
                    load_const(f"wh_{tr}", [128, KT], F32R)
                    load_const(f"bh_{tr}", [1, 1], F32)

            def layer0_mm(tr, g0, mo):
                ps = psum.tile([128, TT], F32, tag="ps")
                nc.tensor.matmul(ps[:], cst[f"w0_{tr}"][:, mo * 128:(mo + 1) * 128],
                                 cst[f"feat_{tr}"][:],
                                 start=True, stop=True)
                nc.scalar.activation(g0[:, mo, :], ps[:], GELU)

            def layer0(tr, tag="acts", bufs=None):
                kw = {} if bufs is None else {"bufs": bufs}
                g0 = acts.tile([128, KT, TT], F32R, tag=tag, **kw)
                for mo in range(KT):
                    layer0_mm(tr, g0, mo)
                return g0

            def head_mm(tr, psh, ko):
                nc.tensor.matmul(psh[:], cst[f"wh_{tr}"][:, ko:ko + 1],
                                 g2s[tr][:, ko, :],
                                 start=(ko == 0), stop=(ko == KT - 1))

            def chain(ps, wslab, g_in, ko_rng, start, stop):
                for ko in ko_rng:
                    nc.tensor.matmul(ps[:], wslab[:, ko * 128:(ko + 1) * 128],
                                     g_in[:, ko, :],
                                     start=(start and ko == ko_rng[0]),
                                     stop=(stop and ko == ko_rng[-1]))

            def load_slab(ly, tr, mo):
                wslab = wpool.tile([128, H], F32R, tag="wslab")
                nc.sync.dma_start(wslab[:], dram[f"w{ly}_{tr}"][mo, :, :])
                return wslab

            def evict(g_out, tr, ly, mo, ps):
                nc.scalar.activation(g_out[:, mo, :], ps[:], GELU,
                                     bias=cst[f"b{ly}_{tr}"][:, mo:mo + 1])

            def layer(tr, ly, g_in, head=False, il_l0=None, warm=0,
                      after_warm=None):
                # head: interleave this trunk's head matmuls (lag 2).
                # il_l0: (trunk, g0) whose layer-0 work rides along this layer.
                # warm: start this many half-chains before g_in fully evicted.
                g_out = acts.tile([128, KT, TT], F32R, tag="acts")
                if head:
                    g2s[tr] = g_out
                    psh = psum_h.tile([1, TT], F32, tag="psh")
                KH = KT // 2
                pend = []
                for mo in range(warm):
                    wslab = load_slab(ly, tr, mo)
                    ps = psum.tile([128, TT], F32, tag="ps")
                    chain(ps, wslab, g_in, range(KH), start=True, stop=False)
                    pend.append((mo, wslab, ps))
                if after_warm is not None:
                    after_warm()
                for mo, wslab, ps in pend:
                    chain(ps, wslab, g_in, range(KH, KT), start=False, stop=True)
                    evict(g_out, tr, ly, mo, ps)
                il_pend = list(range(KT)) if il_l0 is not None else []
                for mo in range(warm, KT):
                    wslab = load_slab(ly, tr, mo)
                    if head and mo >= 2:
                        head_mm(tr, psh, mo - 2)
                    n_il = -(-len(il_pend) // max(1, KT - mo))  # spread evenly
                    for _ in range(n_il):
                        layer0_mm(il_l0[0], il_l0[1], il_pend.pop(0))
                    ps = psum.tile([128, TT], F32, tag="ps")
                    chain(ps, wslab, g_in, range(KT), start=True, stop=True)
                    evict(g_out, tr, ly, mo, ps)
                if head:
                    head_mm(tr, psh, KT - 2)
                    head_mm(tr, psh, KT - 1)
                    rf_sb = outp.tile([1, TT], F32, tag="rf")
                    nc.scalar.activation(rf_sb[:], psh[:], COPY,
                                         bias=cst[f"bh_{tr}"][:1, :1])
                    nc.vector.tensor_scalar(rf_sb[:], rf_sb[:], CLAMP_LO, CLAMP_HI,
                                            op0=mybir.AluOpType.max,
                                            op1=mybir.AluOpType.min)
                    nc.sync.dma_start(dram[f"rf_{tr}"][:], rf_sb[:])
                return g_out

            g2s = {}
            # Pre-warm the PE clock: dependency-free dummy matmuls sized to
            # finish right as layer 0's operands land from HBM.
            warmup = const.tile([128, 64], F32, tag="warmup")
            nc.vector.memset(warmup[:], 0.0)
            ps_w = psum_h.tile([1, TT], F32, tag="psh")
            for _ in range(10):
                nc.tensor.matmul(ps_w[:, 0:64], warmup[:, 0:1], warmup[:],
                                 start=True, stop=True)
            load_trunk_consts("f")
            g0f = layer0("f")
            # g0b has its own slot: it stays live across the whole fwd trunk
            g0b = acts.tile([128, KT, TT], F32R, tag="acts_b0", bufs=1)

            def _deferred_consts():
                load_trunk_consts("f", critical=False)
                load_trunk_consts("b")
                load_trunk_consts("b", critical=False)

            g1f = layer("f", 1, g0f, il_l0=("b", g0b), warm=4,
                        after_warm=_deferred_consts)
            layer("f", 2, g1f, head=True)
            g1b = layer("b", 1, g0b)
            layer("b", 2, g1b, head=True)

    nc.compile()
    return nc


def _compress_w0(W0, b0, mol_vec, solvent_vecs):
    """Fold the feature structure into W0: return (W0effT [F_DIM, H], c [H])."""
    W0 = W0.astype(np.float64)
    cols = []
    for j in range(NSOL):
        off = D_BLOCK * (1 + j)
        cols.append(W0[:, off:off + MOL] @ solvent_vecs[j].astype(np.float64))
    A = W0[:, MOL:MOL + DESC].copy()
    for j in range(NSOL):
        off = D_BLOCK * (1 + j) + MOL
        A += W0[:, off:off + DESC]
    for d in range(DESC):
        cols.append(A[:, d])
    cols.append(W0[:, D_IN - 1])  # prev/next rf column
    W0eff = np.stack(cols, axis=1)  # [H, 12]
    c = W0[:, :MOL] @ mol_vec.astype(np.float64) + b0.astype(np.float64)
    W0effT = np.zeros((F_DIM, H), np.float32)
    W0effT[:W0eff.shape[1], :] = W0eff.T.astype(np.float32)
    return W0effT, c.astype(np.float32)


def _tile_w(W):
    """[H_out, H_in] -> [mo, ki, ko*mi] slabs, lhsT[k, m] = W[m, k]."""
    a = W.reshape(KT, 128, KT, 128)  # [mo, mi, ko, ki]
    return np.ascontiguousarray(a.transpose(0, 3, 2, 1)).reshape(KT, 128, H)


def _part_major(v):
    """[H] -> [128, KT] with v[mo*128+p] at [p, mo]."""
    return np.ascontiguousarray(v.reshape(KT, 128).T)


def kernel(mol_vec, solvent_seq, desc_seq, rf_true, solvent_vecs,
           Wf0, bf0, Wf1, bf1, Wf2, bf2, Wof, bof,
           Wb0, bb0, Wb1, bb1, Wb2, bb2, Wob, bob):
    args = {k: np.asarray(v, np.float32) for k, v in locals().items()}
    mol_vec = args["mol_vec"]; solvent_seq = args["solvent_seq"]
    desc_seq = args["desc_seq"]; rf_true = args["rf_true"]
    solvent_vecs = args["solvent_vecs"]

    prev = np.concatenate([[np.float32(BOUNDARY_RF)], rf_true[:-1]])
    nxt = np.concatenate([rf_true[1:], [np.float32(BOUNDARY_RF)]])

    shared = {}
    for tr, W0, b0, W1, b1, W2, b2, Wh, bh, rf_nb in (
        ("f", args["Wf0"], args["bf0"], args["Wf1"], args["bf1"],
         args["Wf2"], args["bf2"], args["Wof"], args["bof"], prev),
        ("b", args["Wb0"], args["bb0"], args["Wb1"], args["bb1"],
         args["Wb2"], args["bb2"], args["Wob"], args["bob"], nxt),
    ):
        W0effT, c = _compress_w0(W0, b0, mol_vec, solvent_vecs)
        W0effT[NSOL + DESC + 1, :] = c  # bias via the ones-feature row
        w0p = np.zeros((128, H), np.float32)
        w0p[:F_DIM] = W0effT
        shared[f"w0_{tr}"] = w0p
        shared[f"w1_{tr}"] = _tile_w(W1)
        shared[f"b1_{tr}"] = _part_major(b1)
        shared[f"w2_{tr}"] = _tile_w(W2)
        shared[f"b2_{tr}"] = _part_major(b2)
        shared[f"wh_{tr}"] = np.ascontiguousarray(Wh.reshape(KT, 128).T)
        shared[f"bh_{tr}"] = bh.reshape(1, 1)
        feat = np.zeros((128, T), np.float32)
        feat[0:NSOL, :] = solvent_seq.T
        feat[NSOL:NSOL + DESC, :] = desc_seq.T
        feat[NSOL + DESC, :] = rf_nb
        feat[NSOL + DESC + 1, :] = 1.0  # ones row carries the layer-0 bias
        shared[f"feat_{tr}"] = feat

    if "nc" not in _CACHE:
        _CACHE["nc"] = _build_nc()
    nc = _CACHE["nc"]

    in_maps = []
    for core in range(N_CORES):
        m = {k: v for k, v in shared.items() if not k.startswith("feat_")}
        sl = slice(core * TT, (core + 1) * TT)
        m["feat_f"] = np.ascontiguousarray(shared["feat_f"][:, sl])
        m["feat_b"] = np.ascontiguousarray(shared["feat_b"][:, sl])
        in_maps.append(m)

    res = run_bass_kernel_spmd(nc, in_maps, core_ids=list(range(N_CORES)))
    rf_fwd = np.concatenate([res.results[c]["rf_f"][0] for c in range(N_CORES)])
    rf_bwd = np.concatenate([res.results[c]["rf_b"][0] for c in range(N_CORES)])
    return rf_fwd.astype(np.float32), rf_bwd.astype(np.float32)


# revision 55
# speedup vs baseline: 1.0546x; 1.0009x over previous
"""Trainium2 Bass kernel for nn_KermtDualCausalCv4 (dual-trunk dense MLP).

Strategy:
  * Layer-0 algebraic compression: the [T, 12325] input features are
    structured (broadcast mol_vec, rank-1 solvent blocks s_j[t]*v_j,
    desc_seq repeated 6x, prev/next rf scalar). Folding the structure into
    W0 on the host turns the [H, 12325] first layer into an effective
    [H, 12] matmul plus a per-hidden-unit constant (absorbed into the bias):
        h0[t] = W0eff @ f[t] + c,   f[t] = [solvent_seq[t], desc_seq[t], rf_nb[t]]
    This removes ~413 of the 551 GFLOP the reference performs.
  * 8-way data parallelism over the 4096 tokens (512 tokens per core);
    every core runs both trunks end-to-end. No collectives.
  * All matmuls in f32r (full fp32 data, TensorE 1 cycle/row at N=512,
    ~1.5e-4 relative error per matmul measured on HW).
  * Weights pre-tiled on the host into [mo][ki][ko*mi] slabs so each
    weight DMA is 128 partitions x 8KB contiguous (1 MiB per transfer).
  * Exact (erf-based) GELU + per-partition bias fused into the PSUM->SBUF
    eviction on the scalar (ACT) engine.
"""

import sys
import numpy as np

for _p in ("/opt/trn_rl_repo",):
    if _p not in sys.path:
        sys.path.append(_p)

import concourse.bass as bass  # noqa: E402
import concourse.mybir as mybir  # noqa: E402
import concourse.tile as tile  # noqa: E402
from concourse import bacc  # noqa: E402
from concourse.bass_utils import run_bass_kernel_spmd  # noqa: E402

T = 4096
MOL = 2048
DESC = 6
NSOL = 5
H = 2048
D_BLOCK = MOL + DESC  # 2054
D_IN = D_BLOCK * (1 + NSOL) + 1  # 12325
BOUNDARY_RF = 0.5
CLAMP_LO, CLAMP_HI = 1e-4, 1.0 - 1e-4

N_CORES = 8
TT = T // N_CORES  # 512 tokens per core
KT = H // 128  # 16 k/m tiles per hidden dim
F_DIM = 16  # padded effective feature count (12 used)

F32 = mybir.dt.float32
F32R = mybir.dt.float32r
GELU = mybir.ActivationFunctionType.Gelu
COPY = mybir.ActivationFunctionType.Identity

_CACHE: dict = {}


def _build_nc():
    """Emit the Bass/Tile kernel (identical program for all 8 cores)."""
    nc = bacc.Bacc(None, target_bir_lowering=False)

    dram = {}
    for tr in ("f", "b"):
        # feat/w0 are zero-padded to K=128 so layer-0 matmuls are standard
        # full-contraction matmuls (PE cost is N cycles regardless of K, and
        # LDWEIGHTS pipelines like the main layers).
        dram[f"feat_{tr}"] = nc.dram_tensor(f"feat_{tr}", [128, TT], F32R,
                                            kind="ExternalInput")
        dram[f"w0_{tr}"] = nc.dram_tensor(f"w0_{tr}", [128, H], F32R,
                                          kind="ExternalInput")
        for ly in (1, 2):
            dram[f"w{ly}_{tr}"] = nc.dram_tensor(f"w{ly}_{tr}", [KT, 128, H],
                                                 F32R, kind="ExternalInput")
        dram[f"wh_{tr}"] = nc.dram_tensor(f"wh_{tr}", [128, KT], F32R,
                                          kind="ExternalInput")
        for ly in (1, 2):
            dram[f"b{ly}_{tr}"] = nc.dram_tensor(f"b{ly}_{tr}", [128, KT], F32,
                                                 kind="ExternalInput")
        dram[f"bh_{tr}"] = nc.dram_tensor(f"bh_{tr}", [1, 1], F32,
                                          kind="ExternalInput")
        dram[f"rf_{tr}"] = nc.dram_tensor(f"rf_{tr}", [1, TT], F32,
                                          kind="ExternalOutput")

    with tile.TileContext(nc) as tc:
        with (
            tc.tile_pool(name="const", bufs=1) as const,
            tc.tile_pool(name="acts", bufs=3) as acts,
            tc.tile_pool(name="wpool", bufs=5) as wpool,
            tc.tile_pool(name="psum", bufs=5, space="PSUM") as psum,
            tc.tile_pool(name="psum_h", bufs=2, space="PSUM") as psum_h,
            tc.tile_pool(name="outp", bufs=2) as outp,
        ):
            cst = {}

            def load_const(nm, shp, dt):
                t = const.tile(shp, dt, tag=nm)
                nc.sync.dma_start(t[:], dram[nm][:])
                cst[nm] = t

            def load_trunk_consts(tr, critical=True):
                if critical:
                    # feat/w0 gate the first layer-0 matmul: load them first.
                    # (layer-0 bias rides in the matmul via the ones-feature.)
                    load_const(f"feat_{tr}", [128, TT], F32R)
                    if tr == "f":
                        # split so the first matmuls only wait for their slice
                        t = const.tile([128, H], F32R, tag=f"w0_{tr}",
                                       name=f"w0_{tr}")
                        nc.sync.dma_start(t[:, 0:256], dram[f"w0_{tr}"][:, 0:256])
                        nc.sync.dma_start(t[:, 256:H], dram[f"w0_{tr}"][:, 256:H])
                        cst[f"w0_{tr}"] = t
                    else:
                        load_const(f"w0_{tr}", [128, H], F32R)
                else:
                    for ly in (1, 2):
                        load_const(f"b{ly}_{tr}", [128, KT], F32)
                    load_const(f"wh_{tr}", [128, KT], F32R)
                    load_const(f"bh_{tr}", [1, 1], F32)

            def layer0_mm(tr, g0, mo):
                ps = psum.tile([128, TT], F32, tag="ps")
                nc.tensor.matmul(ps[:], cst[f"w0_{tr}"][:, mo * 128:(mo + 1) * 128],
                                 cst[f"feat_{tr}"][:],
                                 start=True, stop=True)
                nc.scalar.activation(g0[:, mo, :], ps[:], GELU)

            def layer0(tr, tag="acts", bufs=None):
                kw = {} if bufs is None else {"bufs": bufs}
                g0 = acts.tile([128, KT, TT], F32R, tag=tag, **kw)
                for mo in range(KT):
                    layer0_mm(tr, g0, mo)
                return g0

            def head_mm(tr, psh, ko):
                nc.tensor.matmul(psh[:], cst[f"wh_{tr}"][:, ko:ko + 1],
                                 g2s[tr][:, ko, :],
                                 start=(ko == 0), stop=(ko == KT - 1))

            def chain(ps, wslab, g_in, ko_rng, start, stop):
                for ko in ko_rng:
                    nc.tensor.matmul(ps[:], wslab[:, ko * 128:(ko + 1) * 128],
                                     g_in[:, ko, :],
                                     start=(start and ko == ko_rng[0]),
                                     stop=(stop and ko == ko_rng[-1]))

            def load_slab(ly, tr, mo):
                wslab = wpool.tile([128, H], F32R, tag="wslab")
                nc.sync.dma_start(wslab[:], dram[f"w{ly}_{tr}"][mo, :, :])
                return wslab

            def evict(g_out, tr, ly, mo, ps):
                nc.scalar.activation(g_out[:, mo, :], ps[:], GELU,
                                     bias=cst[f"b{ly}_{tr}"][:, mo:mo + 1])

            def layer(tr, ly, g_in, head=False, il_l0=None, warm=0,
                      after_warm=None):
                # head: interleave this trunk's head matmuls (lag 2).
                # il_l0: (trunk, g0) whose layer-0 work rides along this layer.
                # warm: start this many half-chains before g_in fully evicted.
                g_out = acts.tile([128, KT, TT], F32R, tag="acts")
                if head:
                    g2s[tr] = g_out
                    psh = psum_h.tile([1, TT], F32, tag="psh")
                # Warm chains: ko-segments interleaved across `warm` chains,
                # paced to g_in's eviction order, with quarter-slab DMAs so
                # early segments don't wait on full 1 MiB slab transfers.
                SEG = 4
                pend = []
                for mo in range(warm):
                    wslab = wpool.tile([128, H], F32R, tag="wslab",
                                       name=f"wslab_w{mo}")
                    ps = psum.tile([128, TT], F32, tag="ps", name=f"ps_w{mo}")
                    pend.append((mo, wslab, ps))
                for p in range(KT // SEG):  # quarter-slab DMAs, part-major
                    for mo, wslab, ps in pend:
                        nc.sync.dma_start(
                            wslab[:, p * SEG * 128:(p + 1) * SEG * 128],
                            dram[f"w{ly}_{tr}"][mo, :,
                                                p * SEG * 128:(p + 1) * SEG * 128])
                for s in range(0, KT, SEG):
                    for mo, wslab, ps in pend:
                        chain(ps, wslab, g_in, range(s, s + SEG),
                              start=(s == 0), stop=(s == KT - SEG))
                    if s == 0 and after_warm is not None:
                        after_warm()
                for mo, wslab, ps in pend:
                    evict(g_out, tr, ly, mo, ps)
                il_pend = list(range(KT)) if il_l0 is not None else []
                for mo in range(warm, KT):
                    wslab = load_slab(ly, tr, mo)
                    if head and mo >= 2:
                        head_mm(tr, psh, mo - 2)
                    n_il = -(-len(il_pend) // max(1, KT - mo))  # spread evenly
                    for _ in range(n_il):
                        layer0_mm(il_l0[0], il_l0[1], il_pend.pop(0))
                    ps = psum.tile([128, TT], F32, tag="ps")
                    chain(ps, wslab, g_in, range(KT), start=True, stop=True)
                    evict(g_out, tr, ly, mo, ps)
                if head:
                    head_mm(tr, psh, KT - 2)
                    head_mm(tr, psh, KT - 1)
                    rf_sb = outp.tile([1, TT], F32, tag="rf")
                    nc.scalar.activation(rf_sb[:], psh[:], COPY,
                                         bias=cst[f"bh_{tr}"][:1, :1])
                    nc.vector.tensor_scalar(rf_sb[:], rf_sb[:], CLAMP_LO, CLAMP_HI,
                                            op0=mybir.AluOpType.max,
                                            op1=mybir.AluOpType.min)
                    nc.sync.dma_start(dram[f"rf_{tr}"][:], rf_sb[:])
                return g_out

            g2s = {}
            # Pre-warm the PE clock: dependency-free dummy matmuls sized to
            # finish right as layer 0's operands land from HBM.
            warmup = const.tile([128, 64], F32, tag="warmup")
            nc.vector.memset(warmup[:], 0.0)
            ps_w = psum_h.tile([1, TT], F32, tag="psh")
            for _ in range(10):
                nc.tensor.matmul(ps_w[:, 0:64], warmup[:, 0:1], warmup[:],
                                 start=True, stop=True)
            load_trunk_consts("f")
            g0f = layer0("f")
            # g0b has its own slot: it stays live across the whole fwd trunk
            g0b = acts.tile([128, KT, TT], F32R, tag="acts_b0", bufs=1)

            def _deferred_consts():
                load_trunk_consts("f", critical=False)
                load_trunk_consts("b")
                load_trunk_consts("b", critical=False)

            g1f = layer("f", 1, g0f, il_l0=("b", g0b), warm=4,
                        after_warm=_deferred_consts)
            layer("f", 2, g1f, head=True)
            g1b = layer("b", 1, g0b)
            layer("b", 2, g1b, head=True)

    nc.compile()
    return nc


def _compress_w0(W0, b0, mol_vec, solvent_vecs):
    """Fold the feature structure into W0: return (W0effT [F_DIM, H], c [H])."""
    W0 = W0.astype(np.float64)
    cols = []
    for j in range(NSOL):
        off = D_BLOCK * (1 + j)
        cols.append(W0[:, off:off + MOL] @ solvent_vecs[j].astype(np.float64))
    A = W0[:, MOL:MOL + DESC].copy()
    for j in range(NSOL):
        off = D_BLOCK * (1 + j) + MOL
        A += W0[:, off:off + DESC]
    for d in range(DESC):
        cols.append(A[:, d])
    cols.append(W0[:, D_IN - 1])  # prev/next rf column
    W0eff = np.stack(cols, axis=1)  # [H, 12]
    c = W0[:, :MOL] @ mol_vec.astype(np.float64) + b0.astype(np.float64)
    W0effT = np.zeros((F_DIM, H), np.float32)
    W0effT[:W0eff.shape[1], :] = W0eff.T.astype(np.float32)
    return W0effT, c.astype(np.float32)


def _tile_w(W):
    """[H_out, H_in] -> [mo, ki, ko*mi] slabs, lhsT[k, m] = W[m, k]."""
    a = W.reshape(KT, 128, KT, 128)  # [mo, mi, ko, ki]
    return np.ascontiguousarray(a.transpose(0, 3, 2, 1)).reshape(KT, 128, H)


def _part_major(v):
    """[H] -> [128, KT] with v[mo*128+p] at [p, mo]."""
    return np.ascontiguousarray(v.reshape(KT, 128).T)


def kernel(mol_vec, solvent_seq, desc_seq, rf_true, solvent_vecs,
           Wf0, bf0, Wf1, bf1, Wf2, bf2, Wof, bof,
           Wb0, bb0, Wb1, bb1, Wb2, bb2, Wob, bob):
    args = {k: np.asarray(v, np.float32) for k, v in locals().items()}
    mol_vec = args["mol_vec"]; solvent_seq = args["solvent_seq"]
    desc_seq = args["desc_seq"]; rf_true = args["rf_true"]
    solvent_vecs = args["solvent_vecs"]

    prev = np.concatenate([[np.float32(BOUNDARY_RF)], rf_true[:-1]])
    nxt = np.concatenate([rf_true[1:], [np.float32(BOUNDARY_RF)]])

    shared = {}
    for tr, W0, b0, W1, b1, W2, b2, Wh, bh, rf_nb in (
        ("f", args["Wf0"], args["bf0"], args["Wf1"], args["bf1"],
         args["Wf2"], args["bf2"], args["Wof"], args["bof"], prev),
        ("b", args["Wb0"], args["bb0"], args["Wb1"], args["bb1"],
         args["Wb2"], args["bb2"], args["Wob"], args["bob"], nxt),
    ):
        W0effT, c = _compress_w0(W0, b0, mol_vec, solvent_vecs)
        W0effT[NSOL + DESC + 1, :] = c  # bias via the ones-feature row
        w0p = np.zeros((128, H), np.float32)
        w0p[:F_DIM] = W0effT
        shared[f"w0_{tr}"] = w0p
        shared[f"w1_{tr}"] = _tile_w(W1)
        shared[f"b1_{tr}"] = _part_major(b1)
        shared[f"w2_{tr}"] = _tile_w(W2)
        shared[f"b2_{tr}"] = _part_major(b2)
        shared[f"wh_{tr}"] = np.ascontiguousarray(Wh.reshape(KT, 128).T)
        shared[f"bh_{tr}"] = bh.reshape(1, 1)
        feat = np.zeros((128, T), np.float32)
        feat[0:NSOL, :] = solvent_seq.T
        feat[NSOL:NSOL + DESC, :] = desc_seq.T
        feat[NSOL + DESC, :] = rf_nb
        feat[NSOL + DESC + 1, :] = 1.0  # ones row carries the layer-0 bias
        shared[f"feat_{tr}"] = feat

    if "nc" not in _CACHE:
        _CACHE["nc"] = _build_nc()
    nc = _CACHE["nc"]

    in_maps = []
    for core in range(N_CORES):
        m = {k: v for k, v in shared.items() if not k.startswith("feat_")}
        sl = slice(core * TT, (core + 1) * TT)
        m["feat_f"] = np.ascontiguousarray(shared["feat_f"][:, sl])
        m["feat_b"] = np.ascontiguousarray(shared["feat_b"][:, sl])
        in_maps.append(m)

    res = run_bass_kernel_spmd(nc, in_maps, core_ids=list(range(N_CORES)))
    rf_fwd = np.concatenate([res.results[c]["rf_f"][0] for c in range(N_CORES)])
    rf_bwd = np.concatenate([res.results[c]["rf_b"][0] for c in range(N_CORES)])
    return rf_fwd.astype(np.float32), rf_bwd.astype(np.float32)
